# revision 21
# baseline (speedup 1.0000x reference)
"""CTC loss on 8 Trainium2 NeuronCores (Bass/Tile).

Strategy (data parallel, per the sharding hint): batch B=64 is split 8
samples/core. The host gathers each sample's distinct lattice emission rows
(1 blank + 30 labels = 31 "slots" per sample) from log_probs, max-normalizes
them, and ships only those ~2MB (fp16) to the device — never the 170MB
log-prob tensor. Each core runs the CTC forward recurrence in linear space:

  - lattice rows computed as first-order scans over t (tensor_tensor_scan),
  - T split into C=16 chunks mapped to SBUF partitions (lanes = (b, c)),
    cross-chunk carries solved exactly with per-slot transfer matrices G
    built on the PE/ACT from bulk chunk-sum cumulants,
  - per-(sample,chunk) static log offsets (host-estimated via a coarse
    windowed DP) keep all stored values in fp32 range; the stitch algebra
    folds the offsets in exactly, so they do not affect the result.

Each core returns only the 8 lattice rows its samples actually end in
(selected on the PE with a host-supplied one-hot matmul, which is exact):
a (8, 99) f32 output per core. Per-sample losses are reconstructed on host
from that, then averaged (the "all-reduce").

The jax persistent compilation cache is enabled so repeat calls skip the
XLA/NEFF compile step (run_bass_kernel_spmd re-lowers an identical HLO
every call; the cache turns that into a fast deserialize).
"""
import os
import tempfile

import numpy as np

import jax

jax.config.update("jax_compilation_cache_dir",
                  os.path.join(tempfile.gettempdir(), "bass_jax_cache"))
jax.config.update("jax_persistent_cache_min_entry_size_bytes", 0)
jax.config.update("jax_persistent_cache_min_compile_time_secs", 0.0)

import concourse.bacc as bacc
import concourse.tile as tile
from concourse import mybir
from concourse.bass_utils import run_bass_kernel_spmd

F32 = mybir.dt.float32
F16 = mybir.dt.float16
U8 = mybir.dt.uint8

T, B, V, S = 512, 64, 1296, 30
L = 2 * S + 1          # 61 lattice rows
NS = S + 1             # 31 distinct emission slots (slot 0 = blank)
C = 16                 # time chunks  (lanes = 8 local samples x 16 chunks)
TC = T // C            # 32 steps per chunk
NCORES = 8
BLOC = B // NCORES     # 8 samples per core
BLANK = 0
NEG = np.float32(-1e30)
OW = 2 * (TC + 1) + 1 + TC   # packed output width: 2 rows + baseM + cumM = 99

_prog_cache = {}
_targets_cache = {}

_BI = np.arange(128) // C              # lane -> local sample
_CI = np.arange(128) % C               # lane -> chunk
_SLOT = np.array([0 if l % 2 == 0 else (l + 1) // 2 for l in range(L)])


def _slot(l):
    return 0 if l % 2 == 0 else (l + 1) // 2


# --------------------------------------------------------------------------
# host-side prep
# --------------------------------------------------------------------------

def _static_mats():
    """Block tri matrices over lanes (b,c), u8-shipped: same for every core."""
    bi, ci = _BI, _CI
    same_b = bi[:, None] == bi[None, :]
    tric = (same_b & (ci[:, None] <= ci[None, :])).astype(np.uint8)
    trics = (same_b & (ci[:, None] < ci[None, :])).astype(np.uint8)
    ident = np.eye(128, dtype=np.uint8)
    return tric, trics, ident


_STATIC_MATS = _static_mats()


def _prep_targets(targets):
    """Per-targets constants: fused gather index (lane layout) + allow masks."""
    key = targets.tobytes()
    if key in _targets_cache:
        return _targets_cache[key]
    t2 = np.asarray(targets).reshape(B, S).astype(np.int64)
    ext = np.zeros((B, L), dtype=np.int64)
    ext[:, 1::2] = t2
    ext_m2 = np.zeros_like(ext)
    ext_m2[:, 2:] = ext[:, :-2]
    allow_odd = ((ext != BLANK) & (ext != ext_m2))[:, 3::2].astype(np.uint8)
    vrows = np.zeros((B, NS), np.int64)
    vrows[:, 1:] = t2                  # slot s>=1 -> label s-1; slot 0 = blank
    # flat-gather index producing the (b, chunk, slot, t') lane layout
    # directly:  idx[b,c,s,t'] = ((c*TC+t')*B + b)*V + vrows[b,s]
    tt = np.arange(T).reshape(C, TC)
    idx = ((tt[None, :, None, :] * B + np.arange(B)[:, None, None, None]) * V
           + vrows[:, None, :, None]).astype(np.int32)
    # per-core u8-blob prefix: [tric | trics | ident | allow2]  (sel one-hot
    # is appended per call, it depends on input_lengths)
    tric, trics, ident = _STATIC_MATS
    u8pre = [np.concatenate(
        [tric, trics, ident, allow_odd[k * BLOC:(k + 1) * BLOC][_BI]],
        axis=1) for k in range(NCORES)]
    out = (idx, u8pre)
    _targets_cache.clear()
    _targets_cache[key] = out
    return out


_WIN = 2
_NW = T // _WIN
_OFFS = np.array([c * (TC // _WIN) + (TC // _WIN) // 2 for c in range(C)])


def _host_prep_fn(lpf, idx):
    """Jitted XLA-CPU host prep: fused gather into lane layout,
    max-normalize, fp16 cast, and the coarse windowed logsumexp DP that
    estimates per-(b, chunk) log levels Lam."""
    import jax.numpy as jnp
    from jax import lax

    A_lane = jnp.take(lpf, idx)                    # (B,C,NS,TC)
    m_lane = A_lane.max(axis=2)                    # (B,C,TC)
    Z_lane = A_lane - m_lane[:, :, None, :]
    z16 = Z_lane.astype(jnp.float16)
    Zw = Z_lane.reshape(B, C, NS, TC // _WIN, _WIN).sum(axis=4) * np.float32(0.5)
    Zw = Zw.transpose(1, 3, 0, 2).reshape(_NW, B, NS)
    zw = Zw[:, :, _SLOT]                           # (nw, B, L)

    negf = jnp.float32(NEG)
    v0 = jnp.full((B, L), negf).at[:, 0].set(0.0).at[:, 1].set(0.0)

    def step(v, zwi):
        # one blurred lattice step per window carrying the window's full
        # emission mass (2*zwi); level-estimate shift vs. two half-steps is
        # a few nats, well inside the fp32 offset margin the stitch needs
        p1 = jnp.concatenate([jnp.full((B, 1), negf), v[:, :-1]], axis=1)
        p2 = jnp.concatenate([jnp.full((B, 2), negf), v[:, :-2]], axis=1)
        mx = jnp.maximum(jnp.maximum(v, p1), p2)
        s = jnp.exp(v - mx) + jnp.exp(p1 - mx) + jnp.exp(p2 - mx)
        v = mx + jnp.log(s) + (zwi + zwi)
        return v, v.max(axis=1)

    _, lev = lax.scan(step, v0, zw)                # lev (nw, B)
    Lam = lev[_OFFS].T                             # (B, C) chunk-middle levels
    return z16, m_lane, Lam


def _host_prep(log_probs, idx):
    if "fn" not in _prog_cache:
        _prog_cache["fn"] = jax.jit(_host_prep_fn)
    cpu = jax.devices("cpu")[0]
    with jax.default_device(cpu):
        z16, m_lane, Lam = _prog_cache["fn"](log_probs.reshape(-1), idx)
        return (np.asarray(z16), np.asarray(m_lane),
                np.asarray(Lam).astype(np.float32))


# --------------------------------------------------------------------------
# device program (identical for all cores; per-core data differs)
# --------------------------------------------------------------------------

def _build_program():
    nc = bacc.Bacc(None)
    # three consolidated inputs (one per dtype) to minimize per-array
    # PJRT transfer overhead over the axon tunnel
    d_z = nc.declare_dram_parameter("z", [128, NS, TC], F16, isOutput=False)
    # f32 blob columns: [0:TC]=m, [TC]=lam, [TC+1]=e0 seed column
    d_f32 = nc.declare_dram_parameter("f32b", [128, TC + 2], F32, isOutput=False)
    # u8 blob columns: [0:128]=tric, [128:256]=trics, [256:384]=ident,
    # [384:413]=allow2, [413:421]=sel one-hot
    d_u8 = nc.declare_dram_parameter("u8b", [128, 421], U8, isOutput=False)
    out = nc.declare_dram_parameter("out", [BLOC, OW], F32, isOutput=True)

    with tile.TileContext(nc) as tc:
        with (
            tc.tile_pool(name="consts", bufs=1) as consts,
            tc.tile_pool(name="rows", bufs=1) as rowsp,
            tc.tile_pool(name="work", bufs=3) as work,
            tc.tile_pool(name="gpool", bufs=3) as gpool,
            tc.tile_pool(name="gamp", bufs=2) as gamp,
            tc.tile_pool(name="ps", bufs=2, space="PSUM") as ps,
            tc.tile_pool(name="ps1", bufs=1, space="PSUM") as ps1,
        ):
            # ---- const loads (u8 -> f32 converts on the ACT engine) ----
            u8_all = consts.tile([128, 421], U8)
            nc.sync.dma_start(out=u8_all[:], in_=d_u8[:])
            sb_tric = consts.tile([128, 128], F32)
            nc.scalar.copy(sb_tric[:], u8_all[:, 0:128])
            sb_trics = consts.tile([128, 128], F32)
            nc.scalar.copy(sb_trics[:], u8_all[:, 128:256])
            sb_ident = consts.tile([128, 128], F32)
            nc.scalar.copy(sb_ident[:], u8_all[:, 256:384])
            # tribias = (trics - 1) * 1e30  (0 where skip allowed, -1e30 else)
            sb_tribias = consts.tile([128, 128], F32)
            nc.vector.tensor_scalar(
                out=sb_tribias[:], in0=sb_trics[:], scalar1=1.0,
                scalar2=1e30,
                op0=mybir.AluOpType.subtract, op1=mybir.AluOpType.mult)
            sb_allow2 = consts.tile([128, 29], F32)
            nc.scalar.copy(sb_allow2[:], u8_all[:, 384:413])
            sb_sel = consts.tile([128, BLOC], F32)
            nc.scalar.copy(sb_sel[:], u8_all[:, 413:421])

            sb_f32 = consts.tile([128, TC + 2], F32)
            nc.sync.dma_start(out=sb_f32[:], in_=d_f32[:])
            sb_lam = sb_f32[:, TC:TC + 1]
            sb_e0 = consts.tile([128, TC], F32)
            nc.vector.memset(sb_e0[:], 0.0)
            nc.scalar.copy(sb_e0[:, 0:1], sb_f32[:, TC + 1:TC + 2])
            sb_ones = consts.tile([1, 128], F32)
            nc.vector.memset(sb_ones[:], 1.0)
            sb_zeros = consts.tile([128, TC], F32)
            nc.vector.memset(sb_zeros[:], 0.0)

            # ---- normalized emission lanes (host-gathered fp16) ----
            sb_z = consts.tile([128, NS, TC], F16)
            nc.sync.dma_start(out=sb_z[:], in_=d_z[:])

            # ---- normalization cumulants ----
            sb_m = sb_f32[:, 0:TC]
            cumM = consts.tile([128, TC], F32)
            nc.vector.tensor_tensor_scan(
                out=cumM[:], data0=sb_m, data1=sb_zeros[:], initial=0.0,
                op0=mybir.AluOpType.add, op1=mybir.AluOpType.add)
            ps_baseM = ps1.tile([128, 1], F32, tag="bulk")
            nc.tensor.matmul(out=ps_baseM[:], lhsT=sb_trics[:],
                             rhs=cumM[:, TC - 1:TC], start=True, stop=True)
            sb_baseM = consts.tile([128, 1], F32)
            nc.scalar.copy(sb_baseM[:], ps_baseM[:])

            # ---- per-slot chunk sums / levels, in slot groups of 8 ----
            sb_p = consts.tile([128, NS, TC], F32)
            sb_S = consts.tile([128, NS], F32)
            biasvec = consts.tile([128, NS], F32)
            msider = consts.tile([128, NS], F32)
            GRP = 8
            for g0 in range(0, NS, GRP):
                g1 = min(g0 + GRP, NS)
                n = g1 - g0
                nc.vector.tensor_reduce(out=sb_S[:, g0:g1],
                                        in_=sb_z[:, g0:g1, :],
                                        axis=mybir.AxisListType.X,
                                        op=mybir.AluOpType.add)
                nc.scalar.activation(sb_p[:, g0:g1, :], sb_z[:, g0:g1, :],
                                     mybir.ActivationFunctionType.Exp)
                ps_lc = ps1.tile([128, GRP], F32, tag="bulk")
                nc.tensor.matmul(out=ps_lc[:, 0:n], lhsT=sb_tric[:],
                                 rhs=sb_S[:, g0:g1], start=True, stop=True)
                nc.vector.tensor_scalar(
                    out=biasvec[:, g0:g1], in0=ps_lc[:, 0:n], scalar1=-1.0,
                    scalar2=sb_lam,
                    op0=mybir.AluOpType.mult, op1=mybir.AluOpType.add)
                ps_lcs = ps1.tile([128, GRP], F32, tag="bulk2")
                nc.tensor.matmul(out=ps_lcs[:, 0:n], lhsT=sb_trics[:],
                                 rhs=sb_S[:, g0:g1], start=True, stop=True)
                nc.vector.tensor_scalar(
                    out=msider[:, g0:g1], in0=ps_lcs[:, 0:n],
                    scalar1=sb_lam, scalar2=None,
                    op0=mybir.AluOpType.subtract)

            # ---- per-slot G transfer matrices ----
            def build_G(s, pool, tag):
                ps_t = ps1.tile([1, 128], F32, tag="ps_t")
                nc.tensor.transpose(out=ps_t[:], in_=msider[:, s:s + 1],
                                    identity=sb_ident[:])
                stg = work.tile([1, 128], F32, tag="stg")
                nc.scalar.copy(stg[:], ps_t[:])
                psG = ps.tile([128, 128], F32, tag="psG")
                nc.tensor.matmul(out=psG[:], lhsT=sb_ones[:],
                                 rhs=stg[:], start=True, stop=False)
                nc.tensor.matmul(out=psG[:], lhsT=sb_ident[:],
                                 rhs=sb_tribias[:], start=False, stop=True)
                Gt = pool.tile([128, 128], F32, tag=tag)
                nc.scalar.activation(Gt[:], psG[:],
                                     mybir.ActivationFunctionType.Exp,
                                     bias=biasvec[:, s:s + 1])
                return Gt

            G_blank = build_G(0, consts, "Gblank")

            # ---- lattice rows ----
            row_tiles = []
            gam_prev = {}
            for l in range(L):
                s = _slot(l)
                Gt = G_blank if s == 0 else build_G(s, gpool, "G")
                p_l = sb_p[:, s, :]
                if l == 0:
                    src_ap = sb_e0[:]
                elif l == 1:
                    srct = work.tile([128, TC], F32, tag="src")
                    nc.vector.tensor_add(out=srct[:],
                                         in0=row_tiles[0][:, 0:TC],
                                         in1=sb_e0[:])
                    src_ap = srct[:]
                elif l % 2 == 0:
                    src_ap = row_tiles[l - 1][:, 0:TC]
                else:
                    srct = work.tile([128, TC], F32, tag="src")
                    nc.vector.tensor_add(out=srct[:],
                                         in0=row_tiles[l - 1][:, 0:TC],
                                         in1=gam_prev[l - 2][:, 0:TC])
                    src_ap = srct[:]

                loc = work.tile([128, TC], F32, tag="loc")
                nc.vector.tensor_tensor_scan(
                    out=loc[:], data0=src_ap, data1=p_l, initial=0.0,
                    op0=mybir.AluOpType.add, op1=mybir.AluOpType.mult)
                xps = ps.tile([128, 1], F32, tag="xps")
                nc.tensor.matmul(out=xps[:], lhsT=Gt[:],
                                 rhs=loc[:, TC - 1:TC], start=True, stop=True)
                rowl = rowsp.tile([128, TC + 1], F32, tag=f"row{l}")
                nc.vector.tensor_tensor_scan(
                    out=rowl[:, 1:TC + 1], data0=src_ap, data1=p_l,
                    initial=xps[:, 0:1],
                    op0=mybir.AluOpType.add, op1=mybir.AluOpType.mult)
                nc.scalar.copy(rowl[:, 0:1], xps[:, 0:1])
                row_tiles.append(rowl)
                if l % 2 == 1 and l + 2 < L:
                    gaml = gamp.tile([128, TC + 1], F32, tag="gam")
                    nc.scalar.mul(gaml[:], rowl[:],
                                  sb_allow2[:, (l - 1) // 2:(l - 1) // 2 + 1])
                    gam_prev[l] = gaml

            # ---- outputs: one-hot matmul picks each sample's final lane
            # (exact: each PSUM sum has exactly one nonzero product) ----
            ps_out = ps1.tile([BLOC, OW], F32, tag="ps_out")
            nc.tensor.matmul(out=ps_out[:, 0:TC + 1], lhsT=sb_sel[:],
                             rhs=row_tiles[L - 2][:], start=True, stop=True)
            nc.tensor.matmul(out=ps_out[:, TC + 1:2 * TC + 2], lhsT=sb_sel[:],
                             rhs=row_tiles[L - 1][:], start=True, stop=True)
            nc.tensor.matmul(out=ps_out[:, 2 * TC + 2:2 * TC + 3],
                             lhsT=sb_sel[:], rhs=sb_baseM[:],
                             start=True, stop=True)
            nc.tensor.matmul(out=ps_out[:, 2 * TC + 3:OW], lhsT=sb_sel[:],
                             rhs=cumM[:], start=True, stop=True)
            sb_out = consts.tile([BLOC, OW], F32)
            nc.scalar.copy(sb_out[:], ps_out[:])
            nc.sync.dma_start(out=out[:], in_=sb_out[:])
    nc.finalize()
    return nc


# --------------------------------------------------------------------------
# entry point
# --------------------------------------------------------------------------

def kernel(log_probs, targets, input_lengths, target_lengths):
    log_probs = np.asarray(log_probs, dtype=np.float32)
    targets = np.asarray(targets)
    input_lengths = np.asarray(input_lengths).astype(np.int64)
    target_lengths = np.asarray(target_lengths)

    idx, u8pre = _prep_targets(targets)

    # fused gather straight into the (b, chunk, slot, t') lane layout,
    # plus normalization and the Lam level DP — one jitted XLA-CPU call
    z_lane, m_lane, Lam = _host_prep(log_probs, idx)

    if "nc" not in _prog_cache:
        _prog_cache["nc"] = _build_program()
    nc = _prog_cache["nc"]

    # final-frame lane selection per sample (host knows input_lengths)
    tE = input_lengths - 1
    cb, tb = tE // TC, tE % TC

    in_maps = []
    for k in range(NCORES):
        bsl = slice(k * BLOC, (k + 1) * BLOC)
        lamk = Lam[bsl][_BI, _CI].reshape(128, 1).astype(np.float32)
        e0c = np.zeros((128, 1), np.float32)
        e0c[_CI == 0, 0] = np.exp(-Lam[bsl][_BI[_CI == 0], 0])
        f32b = np.concatenate(
            [m_lane[bsl].reshape(128, TC), lamk, e0c], axis=1)
        sel = np.zeros((128, BLOC), np.uint8)
        sel[np.arange(BLOC) * C + cb[bsl], np.arange(BLOC)] = 1
        in_maps.append({
            "z": z_lane[bsl].reshape(128, NS, TC),
            "f32b": f32b,
            "u8b": np.concatenate([u8pre[k], sel], axis=1),
        })

    res = run_bass_kernel_spmd(nc, in_maps, core_ids=list(range(NCORES)))

    # host-side: per-sample loss extraction + mean (the "all-reduce")
    o = np.concatenate([res.results[k]["out"] for k in range(NCORES)],
                       axis=0).astype(np.float64)        # (B, OW)
    bb = np.arange(B)
    j = 1 + tb
    A2 = o[bb, j] + o[bb, TC + 1 + j]
    lnorm = o[:, 2 * TC + 2] + o[bb, 2 * TC + 3 + tb] + Lam[bb, cb]
    with np.errstate(divide="ignore", invalid="ignore"):
        losses = -(np.log(A2) + lnorm)
    bad = (A2 <= 0) | ~np.isfinite(losses) | (losses >= 1e29)
    losses[bad] = 0.0
    result = np.float32(np.mean((losses / target_lengths.astype(np.float64))
                                .astype(np.float32)))
    return np.asarray(result, dtype=np.float32)


# revision 22
# speedup vs baseline: 1.3006x; 1.3006x over previous
"""CTC loss on 8 Trainium2 NeuronCores (Bass/Tile).

Strategy (data parallel, per the sharding hint): batch B=64 is split 8
samples/core. The host gathers each sample's distinct lattice emission rows
(1 blank + 30 labels = 31 "slots" per sample) from log_probs, max-normalizes
them, and ships only those ~2MB (fp16) to the device — never the 170MB
log-prob tensor. Each core runs the CTC forward recurrence in linear space:

  - lattice rows computed as first-order scans over t (tensor_tensor_scan),
  - T split into C=16 chunks mapped to SBUF partitions (lanes = (b, c)),
    cross-chunk carries solved exactly with per-slot transfer matrices G
    built on the PE/ACT from bulk chunk-sum cumulants,
  - per-(sample,chunk) static log offsets (host-estimated via a coarse
    windowed DP) keep all stored values in fp32 range; the stitch algebra
    folds the offsets in exactly, so they do not affect the result.

Each core returns only the 8 lattice rows its samples actually end in
(selected on the PE with a host-supplied one-hot matmul, which is exact):
a (8, 99) f32 output per core. Per-sample losses are reconstructed on host
from that, then averaged (the "all-reduce").

The jax persistent compilation cache is enabled so repeat calls skip the
XLA/NEFF compile step (run_bass_kernel_spmd re-lowers an identical HLO
every call; the cache turns that into a fast deserialize).
"""
import os
import tempfile

import numpy as np

import jax

jax.config.update("jax_compilation_cache_dir",
                  os.path.join(tempfile.gettempdir(), "bass_jax_cache"))
jax.config.update("jax_persistent_cache_min_entry_size_bytes", 0)
jax.config.update("jax_persistent_cache_min_compile_time_secs", 0.0)

import concourse.bacc as bacc
import concourse.tile as tile
from concourse import mybir
from concourse.bass_utils import run_bass_kernel_spmd

F32 = mybir.dt.float32
F16 = mybir.dt.float16
F8 = mybir.dt.float8e4
U8 = mybir.dt.uint8

T, B, V, S = 512, 64, 1296, 30
L = 2 * S + 1          # 61 lattice rows
NS = S + 1             # 31 distinct emission slots (slot 0 = blank)
C = 16                 # time chunks  (lanes = 8 local samples x 16 chunks)
TC = T // C            # 32 steps per chunk
NCORES = 8
BLOC = B // NCORES     # 8 samples per core
BLANK = 0
NEG = np.float32(-1e30)
OW = 2 * (TC + 1) + 1 + TC   # packed output width: 2 rows + baseM + cumM = 99

_prog_cache = {}
_targets_cache = {}

_BI = np.arange(128) // C              # lane -> local sample
_CI = np.arange(128) % C               # lane -> chunk
_SLOT = np.array([0 if l % 2 == 0 else (l + 1) // 2 for l in range(L)])


def _slot(l):
    return 0 if l % 2 == 0 else (l + 1) // 2


# --------------------------------------------------------------------------
# host-side prep
# --------------------------------------------------------------------------

def _static_mats():
    """Block tri matrices over lanes (b,c), u8-shipped: same for every core."""
    bi, ci = _BI, _CI
    same_b = bi[:, None] == bi[None, :]
    tric = (same_b & (ci[:, None] <= ci[None, :])).astype(np.uint8)
    trics = (same_b & (ci[:, None] < ci[None, :])).astype(np.uint8)
    ident = np.eye(128, dtype=np.uint8)
    return tric, trics, ident


_STATIC_MATS = _static_mats()


def _prep_targets(targets):
    """Per-targets constants: fused gather index (lane layout) + allow masks."""
    key = targets.tobytes()
    if key in _targets_cache:
        return _targets_cache[key]
    t2 = np.asarray(targets).reshape(B, S).astype(np.int64)
    ext = np.zeros((B, L), dtype=np.int64)
    ext[:, 1::2] = t2
    ext_m2 = np.zeros_like(ext)
    ext_m2[:, 2:] = ext[:, :-2]
    allow_odd = ((ext != BLANK) & (ext != ext_m2))[:, 3::2].astype(np.uint8)
    vrows = np.zeros((B, NS), np.int64)
    vrows[:, 1:] = t2                  # slot s>=1 -> label s-1; slot 0 = blank
    # flat-gather index producing the (b, chunk, slot, t') lane layout
    # directly:  idx[b,c,s,t'] = ((c*TC+t')*B + b)*V + vrows[b,s]
    tt = np.arange(T).reshape(C, TC)
    idx = ((tt[None, :, None, :] * B + np.arange(B)[:, None, None, None]) * V
           + vrows[:, None, :, None]).astype(np.int32)
    # per-core u8-blob prefix: [tric | trics | ident | allow2]  (sel one-hot
    # is appended per call, it depends on input_lengths)
    tric, trics, ident = _STATIC_MATS
    u8pre = [np.concatenate(
        [tric, trics, ident, allow_odd[k * BLOC:(k + 1) * BLOC][_BI]],
        axis=1) for k in range(NCORES)]
    out = (idx, u8pre)
    _targets_cache.clear()
    _targets_cache[key] = out
    return out


_WIN = 2
_NW = T // _WIN
_OFFS = np.array([c * (TC // _WIN) + (TC // _WIN) // 2 for c in range(C)])


def _host_prep_fn(lpf, idx):
    """Jitted XLA-CPU host prep: fused gather into lane layout,
    max-normalize, fp8 cast, and the coarse windowed logsumexp DP that
    estimates per-(b, chunk) log levels Lam."""
    import jax.numpy as jnp
    from jax import lax

    A_lane = jnp.take(lpf, idx)                    # (B,C,NS,TC)
    m_lane = A_lane.max(axis=2)                    # (B,C,TC)
    Z_lane = A_lane - m_lane[:, :, None, :]
    z16 = Z_lane.astype(jnp.float8_e4m3)
    # Lam must be estimated from the QUANTIZED emissions so host offsets
    # stay consistent with what the device actually integrates
    Zq = z16.astype(jnp.float32)
    Zw = Zq.reshape(B, C, NS, TC // _WIN, _WIN).sum(axis=4) * np.float32(0.5)
    Zw = Zw.transpose(1, 3, 0, 2).reshape(_NW, B, NS)
    zw = Zw[:, :, _SLOT]                           # (nw, B, L)

    negf = jnp.float32(NEG)
    v0 = jnp.full((B, L), negf).at[:, 0].set(0.0).at[:, 1].set(0.0)

    def step(v, zwi):
        # one blurred lattice step per window carrying the window's full
        # emission mass (2*zwi); level-estimate shift vs. two half-steps is
        # a few nats, well inside the fp32 offset margin the stitch needs
        p1 = jnp.concatenate([jnp.full((B, 1), negf), v[:, :-1]], axis=1)
        p2 = jnp.concatenate([jnp.full((B, 2), negf), v[:, :-2]], axis=1)
        mx = jnp.maximum(jnp.maximum(v, p1), p2)
        s = jnp.exp(v - mx) + jnp.exp(p1 - mx) + jnp.exp(p2 - mx)
        v = mx + jnp.log(s) + (zwi + zwi)
        return v, v.max(axis=1)

    _, lev = lax.scan(step, v0, zw)                # lev (nw, B)
    Lam = lev[_OFFS].T                             # (B, C) chunk-middle levels
    return z16, m_lane, Lam


def _host_prep(log_probs, idx):
    if "fn" not in _prog_cache:
        _prog_cache["fn"] = jax.jit(_host_prep_fn)
    cpu = jax.devices("cpu")[0]
    with jax.default_device(cpu):
        z16, m_lane, Lam = _prog_cache["fn"](log_probs.reshape(-1), idx)
        return (np.asarray(z16), np.asarray(m_lane),
                np.asarray(Lam).astype(np.float32))


# --------------------------------------------------------------------------
# device program (identical for all cores; per-core data differs)
# --------------------------------------------------------------------------

def _build_program():
    nc = bacc.Bacc(None)
    # three consolidated inputs (one per dtype) to minimize per-array
    # PJRT transfer overhead over the axon tunnel
    d_z = nc.declare_dram_parameter("z", [128, NS, TC], F8, isOutput=False)
    # f32 blob columns: [0:TC]=m, [TC]=lam, [TC+1]=e0 seed column
    d_f32 = nc.declare_dram_parameter("f32b", [128, TC + 2], F32, isOutput=False)
    # u8 blob columns: [0:128]=tric, [128:256]=trics, [256:384]=ident,
    # [384:413]=allow2, [413:421]=sel one-hot
    d_u8 = nc.declare_dram_parameter("u8b", [128, 421], U8, isOutput=False)
    out = nc.declare_dram_parameter("out", [BLOC, OW], F32, isOutput=True)

    with tile.TileContext(nc) as tc:
        with (
            tc.tile_pool(name="consts", bufs=1) as consts,
            tc.tile_pool(name="rows", bufs=1) as rowsp,
            tc.tile_pool(name="work", bufs=3) as work,
            tc.tile_pool(name="gpool", bufs=3) as gpool,
            tc.tile_pool(name="gamp", bufs=2) as gamp,
            tc.tile_pool(name="ps", bufs=2, space="PSUM") as ps,
            tc.tile_pool(name="ps1", bufs=1, space="PSUM") as ps1,
        ):
            # ---- const loads (u8 -> f32 converts on the ACT engine) ----
            u8_all = consts.tile([128, 421], U8)
            nc.sync.dma_start(out=u8_all[:], in_=d_u8[:])
            sb_tric = consts.tile([128, 128], F32)
            nc.scalar.copy(sb_tric[:], u8_all[:, 0:128])
            sb_trics = consts.tile([128, 128], F32)
            nc.scalar.copy(sb_trics[:], u8_all[:, 128:256])
            sb_ident = consts.tile([128, 128], F32)
            nc.scalar.copy(sb_ident[:], u8_all[:, 256:384])
            # tribias = (trics - 1) * 1e30  (0 where skip allowed, -1e30 else)
            sb_tribias = consts.tile([128, 128], F32)
            nc.vector.tensor_scalar(
                out=sb_tribias[:], in0=sb_trics[:], scalar1=1.0,
                scalar2=1e30,
                op0=mybir.AluOpType.subtract, op1=mybir.AluOpType.mult)
            sb_allow2 = consts.tile([128, 29], F32)
            nc.scalar.copy(sb_allow2[:], u8_all[:, 384:413])
            sb_sel = consts.tile([128, BLOC], F32)
            nc.scalar.copy(sb_sel[:], u8_all[:, 413:421])

            sb_f32 = consts.tile([128, TC + 2], F32)
            nc.sync.dma_start(out=sb_f32[:], in_=d_f32[:])
            sb_lam = sb_f32[:, TC:TC + 1]
            sb_e0 = consts.tile([128, TC], F32)
            nc.vector.memset(sb_e0[:], 0.0)
            nc.scalar.copy(sb_e0[:, 0:1], sb_f32[:, TC + 1:TC + 2])
            sb_ones = consts.tile([1, 128], F32)
            nc.vector.memset(sb_ones[:], 1.0)
            sb_zeros = consts.tile([128, TC], F32)
            nc.vector.memset(sb_zeros[:], 0.0)

            # ---- normalized emission lanes (host-gathered fp16) ----
            sb_z = consts.tile([128, NS, TC], F8)
            nc.sync.dma_start(out=sb_z[:], in_=d_z[:])

            # ---- normalization cumulants ----
            sb_m = sb_f32[:, 0:TC]
            cumM = consts.tile([128, TC], F32)
            nc.vector.tensor_tensor_scan(
                out=cumM[:], data0=sb_m, data1=sb_zeros[:], initial=0.0,
                op0=mybir.AluOpType.add, op1=mybir.AluOpType.add)
            ps_baseM = ps1.tile([128, 1], F32, tag="bulk")
            nc.tensor.matmul(out=ps_baseM[:], lhsT=sb_trics[:],
                             rhs=cumM[:, TC - 1:TC], start=True, stop=True)
            sb_baseM = consts.tile([128, 1], F32)
            nc.scalar.copy(sb_baseM[:], ps_baseM[:])

            # ---- per-slot chunk sums / levels, in slot groups of 8 ----
            sb_p = consts.tile([128, NS, TC], F32)
            sb_S = consts.tile([128, NS], F32)
            biasvec = consts.tile([128, NS], F32)
            msider = consts.tile([128, NS], F32)
            GRP = 8
            for g0 in range(0, NS, GRP):
                g1 = min(g0 + GRP, NS)
                n = g1 - g0
                nc.vector.tensor_reduce(out=sb_S[:, g0:g1],
                                        in_=sb_z[:, g0:g1, :],
                                        axis=mybir.AxisListType.X,
                                        op=mybir.AluOpType.add)
                nc.scalar.activation(sb_p[:, g0:g1, :], sb_z[:, g0:g1, :],
                                     mybir.ActivationFunctionType.Exp)
                ps_lc = ps1.tile([128, GRP], F32, tag="bulk")
                nc.tensor.matmul(out=ps_lc[:, 0:n], lhsT=sb_tric[:],
                                 rhs=sb_S[:, g0:g1], start=True, stop=True)
                nc.vector.tensor_scalar(
                    out=biasvec[:, g0:g1], in0=ps_lc[:, 0:n], scalar1=-1.0,
                    scalar2=sb_lam,
                    op0=mybir.AluOpType.mult, op1=mybir.AluOpType.add)
                ps_lcs = ps1.tile([128, GRP], F32, tag="bulk2")
                nc.tensor.matmul(out=ps_lcs[:, 0:n], lhsT=sb_trics[:],
                                 rhs=sb_S[:, g0:g1], start=True, stop=True)
                nc.vector.tensor_scalar(
                    out=msider[:, g0:g1], in0=ps_lcs[:, 0:n],
                    scalar1=sb_lam, scalar2=None,
                    op0=mybir.AluOpType.subtract)

            # ---- per-slot G transfer matrices ----
            def build_G(s, pool, tag):
                ps_t = ps1.tile([1, 128], F32, tag="ps_t")
                nc.tensor.transpose(out=ps_t[:], in_=msider[:, s:s + 1],
                                    identity=sb_ident[:])
                stg = work.tile([1, 128], F32, tag="stg")
                nc.scalar.copy(stg[:], ps_t[:])
                psG = ps.tile([128, 128], F32, tag="psG")
                nc.tensor.matmul(out=psG[:], lhsT=sb_ones[:],
                                 rhs=stg[:], start=True, stop=False)
                nc.tensor.matmul(out=psG[:], lhsT=sb_ident[:],
                                 rhs=sb_tribias[:], start=False, stop=True)
                Gt = pool.tile([128, 128], F32, tag=tag)
                nc.scalar.activation(Gt[:], psG[:],
                                     mybir.ActivationFunctionType.Exp,
                                     bias=biasvec[:, s:s + 1])
                return Gt

            G_blank = build_G(0, consts, "Gblank")

            # ---- lattice rows ----
            row_tiles = []
            gam_prev = {}
            for l in range(L):
                s = _slot(l)
                Gt = G_blank if s == 0 else build_G(s, gpool, "G")
                p_l = sb_p[:, s, :]
                if l == 0:
                    src_ap = sb_e0[:]
                elif l == 1:
                    srct = work.tile([128, TC], F32, tag="src")
                    nc.vector.tensor_add(out=srct[:],
                                         in0=row_tiles[0][:, 0:TC],
                                         in1=sb_e0[:])
                    src_ap = srct[:]
                elif l % 2 == 0:
                    src_ap = row_tiles[l - 1][:, 0:TC]
                else:
                    srct = work.tile([128, TC], F32, tag="src")
                    nc.vector.tensor_add(out=srct[:],
                                         in0=row_tiles[l - 1][:, 0:TC],
                                         in1=gam_prev[l - 2][:, 0:TC])
                    src_ap = srct[:]

                loc = work.tile([128, TC], F32, tag="loc")
                nc.vector.tensor_tensor_scan(
                    out=loc[:], data0=src_ap, data1=p_l, initial=0.0,
                    op0=mybir.AluOpType.add, op1=mybir.AluOpType.mult)
                xps = ps.tile([128, 1], F32, tag="xps")
                nc.tensor.matmul(out=xps[:], lhsT=Gt[:],
                                 rhs=loc[:, TC - 1:TC], start=True, stop=True)
                rowl = rowsp.tile([128, TC + 1], F32, tag=f"row{l}")
                nc.vector.tensor_tensor_scan(
                    out=rowl[:, 1:TC + 1], data0=src_ap, data1=p_l,
                    initial=xps[:, 0:1],
                    op0=mybir.AluOpType.add, op1=mybir.AluOpType.mult)
                nc.scalar.copy(rowl[:, 0:1], xps[:, 0:1])
                row_tiles.append(rowl)
                if l % 2 == 1 and l + 2 < L:
                    gaml = gamp.tile([128, TC + 1], F32, tag="gam")
                    nc.scalar.mul(gaml[:], rowl[:],
                                  sb_allow2[:, (l - 1) // 2:(l - 1) // 2 + 1])
                    gam_prev[l] = gaml

            # ---- outputs: one-hot matmul picks each sample's final lane
            # (exact: each PSUM sum has exactly one nonzero product) ----
            ps_out = ps1.tile([BLOC, OW], F32, tag="ps_out")
            nc.tensor.matmul(out=ps_out[:, 0:TC + 1], lhsT=sb_sel[:],
                             rhs=row_tiles[L - 2][:], start=True, stop=True)
            nc.tensor.matmul(out=ps_out[:, TC + 1:2 * TC + 2], lhsT=sb_sel[:],
                             rhs=row_tiles[L - 1][:], start=True, stop=True)
            nc.tensor.matmul(out=ps_out[:, 2 * TC + 2:2 * TC + 3],
                             lhsT=sb_sel[:], rhs=sb_baseM[:],
                             start=True, stop=True)
            nc.tensor.matmul(out=ps_out[:, 2 * TC + 3:OW], lhsT=sb_sel[:],
                             rhs=cumM[:], start=True, stop=True)
            sb_out = consts.tile([BLOC, OW], F32)
            nc.scalar.copy(sb_out[:], ps_out[:])
            nc.sync.dma_start(out=out[:], in_=sb_out[:])
    nc.finalize()
    return nc


# --------------------------------------------------------------------------
# entry point
# --------------------------------------------------------------------------

def kernel(log_probs, targets, input_lengths, target_lengths):
    log_probs = np.asarray(log_probs, dtype=np.float32)
    targets = np.asarray(targets)
    input_lengths = np.asarray(input_lengths).astype(np.int64)
    target_lengths = np.asarray(target_lengths)

    idx, u8pre = _prep_targets(targets)

    # fused gather straight into the (b, chunk, slot, t') lane layout,
    # plus normalization and the Lam level DP — one jitted XLA-CPU call
    z_lane, m_lane, Lam = _host_prep(log_probs, idx)

    if "nc" not in _prog_cache:
        _prog_cache["nc"] = _build_program()
    nc = _prog_cache["nc"]

    # final-frame lane selection per sample (host knows input_lengths)
    tE = input_lengths - 1
    cb, tb = tE // TC, tE % TC

    in_maps = []
    for k in range(NCORES):
        bsl = slice(k * BLOC, (k + 1) * BLOC)
        lamk = Lam[bsl][_BI, _CI].reshape(128, 1).astype(np.float32)
        e0c = np.zeros((128, 1), np.float32)
        e0c[_CI == 0, 0] = np.exp(-Lam[bsl][_BI[_CI == 0], 0])
        f32b = np.concatenate(
            [m_lane[bsl].reshape(128, TC), lamk, e0c], axis=1)
        sel = np.zeros((128, BLOC), np.uint8)
        sel[np.arange(BLOC) * C + cb[bsl], np.arange(BLOC)] = 1
        in_maps.append({
            "z": z_lane[bsl].reshape(128, NS, TC),
            "f32b": f32b,
            "u8b": np.concatenate([u8pre[k], sel], axis=1),
        })

    res = run_bass_kernel_spmd(nc, in_maps, core_ids=list(range(NCORES)))

    # host-side: per-sample loss extraction + mean (the "all-reduce")
    o = np.concatenate([res.results[k]["out"] for k in range(NCORES)],
                       axis=0).astype(np.float64)        # (B, OW)
    bb = np.arange(B)
    j = 1 + tb
    A2 = o[bb, j] + o[bb, TC + 1 + j]
    lnorm = o[:, 2 * TC + 2] + o[bb, 2 * TC + 3 + tb] + Lam[bb, cb]
    with np.errstate(divide="ignore", invalid="ignore"):
        losses = -(np.log(A2) + lnorm)
    bad = (A2 <= 0) | ~np.isfinite(losses) | (losses >= 1e29)
    losses[bad] = 0.0
    result = np.float32(np.mean((losses / target_lengths.astype(np.float64))
                                .astype(np.float32)))
    return np.asarray(result, dtype=np.float32)
